# revision 10
# baseline (speedup 1.0000x reference)
"""MultiHeadGraphAttention Trainium2 kernel.

Reference computation (N=6144, HIDDEN=48, HEADS=3, HEAD=16):
    q = (h @ Wq.T + bq)  -> [H, N, 16]   (likewise k, v)
    scores = softmax( (q k^T) * A * sqrt(N), axis=-1 )   # [H, N, N]
    out    = scores @ v  -> [N, 48]
    returns (out, scores)

Strategy: shard the query-row dimension N across 8 NeuronCores (768 rows
each).  The tiny projections (48x48) are done on host; each core receives
q^T (pre-scaled by sqrt(N), heads padded to 32-partition offsets for PE
row-group packing), k^T (same layout), v (natural), its A row-block in
bf16 (exact for a 0/1 mask, halves DMA traffic), and a 128x128 identity
for PE transposes.

Per 128-row block, per head, on device:
  1. PE: 12 fp32 matmuls [K=16] q^T.T @ k^T -> scores chunk in PSUM
     (3 heads packed into PE row groups 0/32/64 -> run concurrently)
  2. DVE: tensor_tensor_reduce: t = psum * A (mask) fused with running
     row-max m (chained across the 12 chunks)
  3. ACT: E = exp(t - m) with per-partition bias, accum_out gives Z free
  4. DVE: recipZ = 1/Z ; P = E * recipZ ; DMA P out (the scores output)
  5. PE: transpose E in 128x128 tiles -> PSUM, copy to SBUF
  6. PE: PV matmuls trE.T @ v accumulated in PSUM -> out rows; scaled by
     recipZ on ACT at the end.
"""

import os
from contextlib import ExitStack

import numpy as np
import ml_dtypes

import concourse.bass as bass
import concourse.tile as tile
from concourse import bacc, mybir
from concourse.bass_utils import run_bass_kernel_spmd

N = 6144
HIDDEN = 48
HEADS = 3
HEAD = 16
NCORES = 8
ROWS = N // NCORES          # 768 rows per core
NBLK = ROWS // 128          # 6 row blocks per core
NCHUNK = N // 512           # 12 key chunks of 512
JBLK = N // 128             # 48 key blocks of 128

F32 = mybir.dt.float32
BF16 = mybir.dt.bfloat16


def build_kernel_nc(pack=True):
    nc = bacc.Bacc("TRN2", target_bir_lowering=False, debug=False)

    qT = nc.dram_tensor("qT", [96, ROWS], F32, kind="ExternalInput").ap()
    kT = nc.dram_tensor("kT", [96, N], F32, kind="ExternalInput").ap()
    v_in = nc.dram_tensor("v", [N, HIDDEN], F32, kind="ExternalInput").ap()
    A_in = nc.dram_tensor("Ab", [ROWS, N], BF16, kind="ExternalInput").ap()
    ident_in = nc.dram_tensor("ident", [128, 128], F32, kind="ExternalInput").ap()
    scores_out = nc.dram_tensor("scores", [HEADS, ROWS, N], F32,
                                kind="ExternalOutput").ap()
    out_out = nc.dram_tensor("outb", [ROWS, HIDDEN], F32,
                             kind="ExternalOutput").ap()

    with tile.TileContext(nc) as tc, ExitStack() as ctx:
        consts = ctx.enter_context(tc.tile_pool(name="consts", bufs=1))
        a_pool = ctx.enter_context(tc.tile_pool(name="a_pool", bufs=2))
        big = ctx.enter_context(tc.tile_pool(name="big", bufs=4))
        tre_pool = ctx.enter_context(tc.tile_pool(name="tre", bufs=2))
        stats = ctx.enter_context(tc.tile_pool(name="stats", bufs=8))
        osb_pool = ctx.enter_context(tc.tile_pool(name="osb", bufs=2))
        qk_ps = ctx.enter_context(tc.tile_pool(name="qk_ps", bufs=5, space="PSUM"))
        tr_ps = ctx.enter_context(tc.tile_pool(name="tr_ps", bufs=2, space="PSUM"))
        pv_ps = ctx.enter_context(tc.tile_pool(name="pv_ps", bufs=1, space="PSUM"))

        # ---- resident tensors --------------------------------------------
        if pack:
            qT_sb = consts.tile([96, ROWS], F32)
            nc.sync.dma_start(qT_sb[:], qT[:])
            kT_sb = consts.tile([96, N], F32)
            nc.sync.dma_start(kT_sb[:], kT[:])
            q_ap = [qT_sb[32 * h:32 * h + HEAD, :] for h in range(HEADS)]
            k_ap = [kT_sb[32 * h:32 * h + HEAD, :] for h in range(HEADS)]
            tpos = [(32 * h, 0) for h in range(HEADS)]
        else:
            q_ap, k_ap = [], []
            tpos = [None] * HEADS
            for h in range(HEADS):
                qh = consts.tile([HEAD, ROWS], F32, name=f"qh{h}")
                nc.sync.dma_start(qh[:], qT[32 * h:32 * h + HEAD, :])
                kh = consts.tile([HEAD, N], F32, name=f"kh{h}")
                nc.sync.dma_start(kh[:], kT[32 * h:32 * h + HEAD, :])
                q_ap.append(qh[:, :])
                k_ap.append(kh[:, :])
        # v natural [j, d] laid out as [128, jblk*48]: block jb at cols 48*jb
        v_sb = consts.tile([128, JBLK * HIDDEN], F32)
        nc.sync.dma_start(
            v_sb[:].rearrange("p (b d) -> p b d", b=JBLK),
            v_in.rearrange("(b p) d -> p b d", p=128))
        ident_sb = consts.tile([128, 128], F32)
        nc.sync.dma_start(ident_sb[:], ident_in[:])

        for blk in range(NBLK):
            r0 = blk * 128
            # A halves (bf16): [128, 3072] each
            a_half = [a_pool.tile([128, N // 2], BF16, tag="ah", name=f"ah{blk}_{i}") for i in range(2)]
            for hf in range(2):
                nc.sync.dma_start(
                    a_half[hf][:],
                    A_in[r0:r0 + 128, hf * (N // 2):(hf + 1) * (N // 2)])

            # ---- QK + mask, chunk-major for head packing ----------------
            t_sb = []
            for h in range(HEADS):
                t_sb.append(big.tile([128, N], F32, tag="big", name=f"t{blk}_{h}"))
            for c in range(NCHUNK):
                ps = []
                for h in range(HEADS):
                    p = qk_ps.tile([128, 512], F32, tag="qk")
                    nc.tensor.matmul(
                        p[:],
                        lhsT=q_ap[h][:, r0:r0 + 128],
                        rhs=k_ap[h][:, c * 512:(c + 1) * 512],
                        start=True, stop=True,
                        tile_position=tpos[h],
                    )
                    ps.append(p)
                hf = 0 if c < NCHUNK // 2 else 1
                cc = c % (NCHUNK // 2)
                for h in range(HEADS):
                    nc.vector.tensor_tensor(
                        out=t_sb[h][:, c * 512:(c + 1) * 512],
                        in0=ps[h][:],
                        in1=a_half[hf][:, cc * 512:(cc + 1) * 512],
                        op=mybir.AluOpType.mult,
                    )

            # ---- per-head softmax + PV ----------------------------------
            osb = osb_pool.tile([128, HIDDEN], F32, tag="osb")
            pv = pv_ps.tile([128, HIDDEN], F32, tag="pv")
            for h in range(HEADS):
                negm = stats.tile([128, 1], F32, tag="st2")
                nc.vector.reduce_max(
                    negm[:], t_sb[h][:], axis=mybir.AxisListType.X,
                    negate=True)
                z = stats.tile([128, 1], F32, tag="st3")
                e_sb = big.tile([128, N], F32, tag="big")
                nc.scalar.activation(
                    e_sb[:], t_sb[h][:],
                    mybir.ActivationFunctionType.Exp,
                    bias=negm[:], scale=1.0, accum_out=z[:])
                rz = stats.tile([128, 1], F32, tag="st4")
                nc.vector.reciprocal(rz[:], z[:])

                # normalized probabilities -> HBM (scores output)
                p_sb = big.tile([128, N], F32, tag="big")
                norm_eng = os.environ.get("GQA_NORM", "dve")
                if norm_eng == "pool":
                    nc.gpsimd.tensor_scalar_mul(p_sb[:], e_sb[:], rz[:])
                elif norm_eng == "act":
                    nc.scalar.mul(p_sb[:], e_sb[:], rz[:])
                else:
                    nc.vector.tensor_scalar_mul(p_sb[:], e_sb[:], rz[:])
                nc.sync.dma_start(scores_out[h, r0:r0 + 128, :], p_sb[:])

                # PV: transpose E tile-by-tile on PE, then matmul with v
                for jb4 in range(JBLK // 4):
                    trp = tr_ps.tile([128, 512], F32, tag="trp")
                    for k in range(4):
                        jb = jb4 * 4 + k
                        nc.tensor.transpose(
                            trp[:, k * 128:(k + 1) * 128],
                            e_sb[:, jb * 128:(jb + 1) * 128],
                            ident_sb[:])
                    tre = tre_pool.tile([128, 512], F32, tag="tre")
                    if jb4 % 2 == 0:
                        nc.vector.tensor_copy(tre[:], trp[:])
                    else:
                        nc.scalar.copy(tre[:], trp[:])
                    for k in range(4):
                        jb = jb4 * 4 + k
                        nc.tensor.matmul(
                            pv[:, HEAD * h:HEAD * (h + 1)],
                            lhsT=tre[:, k * 128:(k + 1) * 128],
                            rhs=v_sb[:, jb * HIDDEN + HEAD * h:
                                     jb * HIDDEN + HEAD * (h + 1)],
                            start=(jb == 0), stop=(jb == JBLK - 1),
                        )
                # out rows = pv * (1/Z)
                nc.scalar.mul(osb[:, HEAD * h:HEAD * (h + 1)],
                              pv[:, HEAD * h:HEAD * (h + 1)], rz[:])
            nc.sync.dma_start(out_out[r0:r0 + 128, :], osb[:])

    nc.compile()
    return nc


_NC_CACHE = None


def _get_nc():
    global _NC_CACHE
    if _NC_CACHE is None:
        _NC_CACHE = build_kernel_nc(
            pack=os.environ.get("GQA_PACK", "1") == "1")
    return _NC_CACHE


def _host_prep(A, h, Wq, bq, Wk, bk, Wv, bv):
    """Host-side projections and input layout for each core."""
    h32 = np.asarray(h, np.float32)
    q = h32 @ np.asarray(Wq, np.float32).T + np.asarray(bq, np.float32)
    k = h32 @ np.asarray(Wk, np.float32).T + np.asarray(bk, np.float32)
    v = h32 @ np.asarray(Wv, np.float32).T + np.asarray(bv, np.float32)
    q *= np.float32(np.sqrt(np.float32(N)))

    # head-padded transposed layouts [96, N]: head hd at partitions 32*hd
    qT = np.zeros((96, N), np.float32)
    kTp = np.zeros((96, N), np.float32)
    for hd in range(HEADS):
        qT[32 * hd:32 * hd + HEAD] = q[:, HEAD * hd:HEAD * (hd + 1)].T
        kTp[32 * hd:32 * hd + HEAD] = k[:, HEAD * hd:HEAD * (hd + 1)].T

    A_bf = np.asarray(A, np.float32).astype(ml_dtypes.bfloat16)
    ident = np.eye(128, dtype=np.float32)

    in_maps = []
    for c in range(NCORES):
        in_maps.append({
            "qT": np.ascontiguousarray(qT[:, c * ROWS:(c + 1) * ROWS]),
            "kT": np.ascontiguousarray(kTp),
            "v": np.ascontiguousarray(v),
            "Ab": np.ascontiguousarray(A_bf[c * ROWS:(c + 1) * ROWS]),
            "ident": ident,
        })
    return in_maps


def kernel(A, h, Wq, bq, Wk, bk, Wv, bv, _trace=False):
    nc = _get_nc()
    in_maps = _host_prep(A, h, Wq, bq, Wk, bk, Wv, bv)
    res = run_bass_kernel_spmd(nc, in_maps, core_ids=list(range(NCORES)),
                               trace=_trace)
    scores = np.concatenate([res.results[c]["scores"] for c in range(NCORES)],
                            axis=1)
    out = np.concatenate([res.results[c]["outb"] for c in range(NCORES)],
                         axis=0)
    if _trace:
        kernel.last_exec_time_ns = res.exec_time_ns
        kernel.last_results = res
    return (out, scores)
